# revision 28
# baseline (speedup 1.0000x reference)
"""Trainium2 Bass kernel for batch-8 multi-head attention (B=8, N=1024, C=768, H=12).

Distribution: pure data parallelism — batch element i runs entirely on core i
(weights replicated, zero collectives, full inputs sharded on host).

Host-side prep inside kernel(): inputs cast to bf16, x pre-transposed per
core, bias pre-broadcast to [128, C], so xT / W_qkv / W_proj / b DMA straight
into their SBUF layouts with no on-device conversion or PE transposes; the
output DMAs out as bf16.

Per-core pipeline:
  xT[k, t]      DMA'd directly (host-transposed), chunk k = cols [k*N, ...)
  qT/kT         W_qkv col-block (stationary) x xT (moving) -> 24 per-head
                blocks [128, N]; rows 64:127 zeroed (Pool memsets) so scores
                stream a full 128-wide contraction (both operands must be
                NaN-free there: 0*garbage = NaN)
  v[t, c]       xT chunk (stationary) x W_v (moving), stored as H blocks of
                [v_h(64) | 1] per s-tile (v65 memset to 1.0 once)
  ST[s, t]      kT slice (stationary) x qT (moving)           (scores^T)
  expST         ACT exp(SCALE * ST) PSUM->SBUF bf16; ACT does almost nothing
                else so the 96 exps stream densely
  y65[t, d|cs]  expST slice (stationary) x [v|1] (moving); col 64 = softmax
                denominator -> DVE reciprocal + tensor_scalar normalize
  yT            PE transposes of y_nat (only transposes left on the PE)
  z[t, c]       split-K projection: chunks 0-2 (heads 0-5) projected during
                heads 9-12 into z1 = psum + bias; chunks 3-5 at the end,
                z = psum + z1, DMA'd per t-tile as soon as ready

Emission interleaves scores s-tiles of head h with AV t-tiles of head h-3
(exp pool = 24 tiles = 3 heads) plus one late qk column per head, keeping
the in-order PE queue from parking useful work behind an exp-paced wait.
"""
import numpy as np
import ml_dtypes

import concourse.bacc as bacc
import concourse.bass as bass
import concourse.tile as tile
import concourse.mybir as mybir
from concourse import masks
from concourse.bass_utils import run_bass_kernel_spmd

F32 = mybir.dt.float32
BF16 = mybir.dt.bfloat16

B, N, C = 8, 1024, 768
H, D = 12, 64
SCALE = float(D) ** -0.5
N_CORES = 8
KT = C // 128             # 6 contraction chunks of 128
TT = N // 128             # 8 token tiles of 128
ST = N // 128             # 8 key tiles of 128
EXP_FN = mybir.ActivationFunctionType.Exp


def halves(width):
    out = []
    off = 0
    while off < width:
        w = min(512, width - off)
        out.append((off, w))
        off += w
    return out


def build_nc():
    nc = bacc.Bacc("TRN2", target_bir_lowering=False, debug=False,
                   num_devices=N_CORES)
    xt_ext = nc.dram_tensor("xT", [C, N], BF16, kind="ExternalInput")
    wqkv_ext = nc.dram_tensor("W_qkv", [C, 3 * C], BF16, kind="ExternalInput")
    wproj_ext = nc.dram_tensor("W_proj", [C, C], BF16, kind="ExternalInput")
    bb_ext = nc.dram_tensor("b_bc", [128, C], F32, kind="ExternalInput")
    out_ext = nc.dram_tensor("out", [N, C], BF16, kind="ExternalOutput")

    with tile.TileContext(nc) as tc:
        with (
            tc.tile_pool(name="const", bufs=1) as constp,
            tc.tile_pool(name="wq", bufs=1) as wqp,
            tc.tile_pool(name="xt", bufs=1) as xtp,
            tc.tile_pool(name="qk", bufs=1) as qkp,
            tc.tile_pool(name="vp", bufs=1) as vp,
            tc.tile_pool(name="yt", bufs=1) as ytp,
            tc.tile_pool(name="yn", bufs=1) as ynp,
            tc.tile_pool(name="z1p", bufs=1) as z1p,
            tc.tile_pool(name="recip", bufs=1) as recipp,
            tc.tile_pool(name="exp", bufs=24) as expp,
            tc.tile_pool(name="z", bufs=2) as zp,
            tc.tile_pool(name="psum", bufs=3, space="PSUM") as psum,
        ):
            # ---- persistent tensors ----
            xt_bf = xtp.tile([128, KT * N], BF16)          # xT: chunk k at cols [k*N, ...)
            wq_bf = wqp.tile([128, KT * 3 * C], BF16)      # W_qkv chunk k at cols [k*3C, ...)
            wp_bf = wqp.tile([128, KT * C], BF16)          # W_proj chunk k at cols [k*C, ...)
            qk_pad = qkp.tile([128, 24 * N], BF16)         # qT blocks 0..11, kT blocks 12..23
            v65 = vp.tile([128, ST * H * 65], BF16)        # per s-tile: H blocks of [v_h(64)|1]
            y_nat = ynp.tile([128, TT * C], BF16)          # y natural: t-tile t at cols [t*C, ...)
            yt_bf = ytp.tile([128, KT * N], BF16)          # yT: chunk c at cols [c*N, ...)
            z1_sb = z1p.tile([128, TT * C], BF16)          # proj half 1 partial (incl bias)

            # ---- constants ----
            ident = constp.tile([128, 128], BF16)
            masks.make_identity(nc, ident[:])
            b_bcast = constp.tile([128, C], F32)

            # ---- DMAs: xT + W_proj (+ outputs later) on sync queue, W_qkv
            # + bias on the scalar queue ----
            for k in range(KT):
                nc.sync.dma_start(xt_bf[:, k * N:(k + 1) * N],
                                  xt_ext[k * 128:(k + 1) * 128, :])
            for k in range(KT):
                nc.scalar.dma_start(wq_bf[:, k * 3 * C:(k + 1) * 3 * C],
                                    wqkv_ext[k * 128:(k + 1) * 128, :])
            for k in range(KT):
                nc.sync.dma_start(wp_bf[:, k * C:(k + 1) * C],
                                  wproj_ext[k * 128:(k + 1) * 128, :])
            nc.scalar.dma_start(b_bcast[:], bb_ext[:])

            # ---- zero pad rows + v65 ones (Pool; SBUF-only engine) ----
            # blocks for heads 0/1 first so scores h0/h1 aren't gated on the
            # big sweep
            nc.gpsimd.memset(qk_pad[64:128, 0:2 * N], 0.0)
            nc.gpsimd.memset(qk_pad[64:128, 12 * N:14 * N], 0.0)
            nc.gpsimd.memset(v65[:], 1.0)
            nc.gpsimd.memset(qk_pad[64:128, 2 * N:7 * N], 0.0)
            nc.gpsimd.memset(qk_pad[64:128, 14 * N:19 * N], 0.0)
            nc.gpsimd.memset(qk_pad[64:128, 7 * N:12 * N], 0.0)
            nc.gpsimd.memset(qk_pad[64:128, 19 * N:24 * N], 0.0)

            # ---- emitters ----
            def emit_qk_col(co, ev0, ev1):
                # co 0..5: q pair (heads 2co, 2co+1) -> blocks 2co, 2co+1
                # co 6..11: k pair -> blocks 12+2(co-6), 13+2(co-6)
                woff = co * 128 if co < KT else C + (co - KT) * 128
                qk_ps = psum.tile([128, N], F32, tag="ps")
                for k in range(KT):
                    lhsT = wq_bf[:, k * 3 * C + woff: k * 3 * C + woff + 128]
                    for off, w in halves(N):
                        nc.tensor.matmul(qk_ps[:, off:off + w], lhsT,
                                         xt_bf[:, k * N + off: k * N + off + w],
                                         start=(k == 0), stop=(k == KT - 1))
                if co < KT:
                    blk0, blk1 = 2 * co, 2 * co + 1
                else:
                    blk0, blk1 = 12 + 2 * (co - KT), 13 + 2 * (co - KT)
                ev0(qk_pad[0:64, blk0 * N:(blk0 + 1) * N], qk_ps[0:64, :])
                ev1(qk_pad[0:64, blk1 * N:(blk1 + 1) * N], qk_ps[64:128, :])

            def emit_v_tile(t):
                v_ps = psum.tile([128, C], F32, tag="ps")
                for k in range(KT):
                    lhsT = xt_bf[:, k * N + t * 128: k * N + (t + 1) * 128]
                    for off, w in halves(C):
                        nc.tensor.matmul(v_ps[:, off:off + w], lhsT,
                                         wq_bf[:, k * 3 * C + 2 * C + off: k * 3 * C + 2 * C + off + w],
                                         start=(k == 0), stop=(k == KT - 1))
                base = t * H * 65
                v_view = v65[:, base: base + H * 65].rearrange("p (h w) -> p h w", w=65)
                nc.vector.tensor_copy(v_view[:, :, 0:64],
                                      v_ps[:].rearrange("p (h d) -> p h d", d=64))

            e_tiles = {}

            def emit_score(h, s):
                k_ap = qk_pad[:, (12 + h) * N:(12 + h + 1) * N]
                q_ap = qk_pad[:, h * N:(h + 1) * N]
                s_ps = psum.tile([128, N], F32, tag="ps")
                for off, w in halves(N):
                    nc.tensor.matmul(s_ps[:, off:off + w],
                                     k_ap[:, s * 128:(s + 1) * 128],
                                     q_ap[:, off:off + w],
                                     start=True, stop=True)
                e_t = expp.tile([128, N], BF16, tag="exp")
                nc.scalar.activation(e_t[:], s_ps[:], EXP_FN, bias=0.0, scale=SCALE)
                e_tiles.setdefault(h, []).append(e_t)

            def emit_av(h, t):
                tiles = e_tiles[h]
                y_ps = psum.tile([128, 65], F32, tag="tp", bufs=2)
                for s in range(ST):
                    lhsT = tiles[s][:, t * 128:(t + 1) * 128]
                    rhs = v65[:, s * H * 65 + h * 65: s * H * 65 + (h + 1) * 65]
                    nc.tensor.matmul(y_ps[:, 0:65], lhsT, rhs,
                                     start=(s == 0), stop=(s == ST - 1))
                recip = recipp.tile([128, 1], F32, tag="recip", bufs=4)
                nc.vector.reciprocal(recip[:, 0:1], y_ps[:, 64:65])
                dst = y_nat[:, t * C + h * 64: t * C + (h + 1) * 64]
                nc.vector.tensor_scalar_mul(dst, y_ps[:, 0:64], recip[:, 0:1])
                if t == TT - 1:
                    e_tiles.pop(h)

            def emit_ytrans(i):
                # transpose y_nat c-chunk i (heads 2i, 2i+1) into yt_bf
                tp_ps = psum.tile([128, N], BF16, tag="tp", bufs=2)
                for t in range(TT):
                    nc.tensor.transpose(
                        tp_ps[:, t * 128:(t + 1) * 128],
                        y_nat[:, t * C + i * 128: t * C + (i + 1) * 128],
                        ident[:])
                nc.vector.tensor_copy(yt_bf[:, i * N:(i + 1) * N], tp_ps[:])

            def emit_proj(t, ks, mode):
                z_ps = psum.tile([128, C], F32, tag="ps")
                for k in ks:
                    lhsT = yt_bf[:, k * N + t * 128: k * N + (t + 1) * 128]
                    for off, w in halves(C):
                        nc.tensor.matmul(z_ps[:, off:off + w], lhsT,
                                         wp_bf[:, k * C + off: k * C + off + w],
                                         start=(k == ks[0]), stop=(k == ks[-1]))
                z1 = z1_sb[:, t * C:(t + 1) * C]
                if mode == 0:          # z1 = psum + bias
                    nc.vector.tensor_add(z1, z_ps[:], b_bcast[:])
                elif mode == 1:        # z1 += psum (in place)
                    nc.vector.tensor_add(z1, z_ps[:], z1)
                else:                  # z = z1 + psum -> bf16 -> out
                    z_sb = zp.tile([128, C], BF16, tag="z")
                    nc.vector.tensor_add(z_sb[:], z_ps[:], z1)
                    nc.sync.dma_start(out_ext[t * 128:(t + 1) * 128, :], z_sb[:])

            # ---- qkv warm-up: cols for heads 0-3, scores h0-h2, v tiles ----
            _qs = nc.enter_named_scope("qkv", False)
            V = nc.vector.tensor_copy
            A = nc.scalar.copy
            emit_qk_col(0, A, A)
            emit_qk_col(6, A, A)
            for s in range(ST):
                emit_score(0, s)
            emit_qk_col(1, V, V)
            emit_qk_col(7, V, V)
            for s in range(ST):
                emit_score(1, s)
            for t in range(0, 3):
                emit_v_tile(t)
            for s in range(ST):
                emit_score(2, s)
            for t in range(3, 6):
                emit_v_tile(t)
            nc.leave_named_scope("qkv", _qs[0], False)

            # ---- attention: interleaved scores(h) / AV(h-3) / extras ----
            # (exp pool = 24 tiles = 3 heads, so AV trails scores by 3)
            _as = nc.enter_named_scope("attn", False)
            emit_v_tile(6)
            emit_v_tile(7)
            emit_qk_col(2, V, V)
            late_cols = [8, 3, 9, 4, 10, 5, 11]
            proj1_t = 0
            for h in range(3, H + 2):
                hs = h if h < H else None
                avs = [0, 1] if h == 3 else [h - 2]
                av_pairs = [(a, t) for a in avs for t in range(TT)]
                extras = []
                if h - 3 < len(late_cols):
                    extras.append(("col", late_cols[h - 3]))
                if h >= 8 and proj1_t < TT:
                    extras.append(("p", proj1_t))
                    extras.append(("p", proj1_t + 1))
                    proj1_t += 2
                if h in (10, 11):      # chunk 3 once yt3 exists (h=9)
                    extras += [("p3", t) for t in range(0 if h == 10 else 4,
                                                      4 if h == 10 else 8)]
                # ytrans(i) once AV for head pair (2i, 2i+1) is done
                yt_is = [a // 2 for a in avs if a % 2 == 1]

                def run_extra(e):
                    kind, arg = e
                    if kind == "col":
                        emit_qk_col(arg, V, V)
                    elif kind == "p":
                        emit_proj(arg, (0, 1, 2), 0)
                    else:
                        emit_proj(arg, (3,), 1)

                ai = iter(av_pairs)

                def take2():
                    for p in (next(ai, None), next(ai, None)):
                        if p:
                            emit_av(*p)

                if hs is not None:
                    for s in range(3):
                        emit_score(hs, s)
                    take2()
                    emit_score(hs, 3)
                    take2()
                    emit_score(hs, 4)
                    take2()
                    emit_score(hs, 5)
                    if extras:
                        run_extra(extras.pop(0))
                    emit_score(hs, 6)
                    take2()
                    emit_score(hs, 7)
                    for p in ai:
                        emit_av(*p)
                    for e in extras:
                        run_extra(e)
                else:
                    for p in ai:
                        emit_av(*p)
                    for e in extras:
                        run_extra(e)
                for i in yt_is:
                    emit_ytrans(i)
            nc.leave_named_scope("attn", _as[0], False)

            # ---- projection tail: chunks 4-5 + output ----
            _ps_ = nc.enter_named_scope("proj", False)
            for t in range(TT):
                emit_proj(t, (4, 5), 2)
            nc.leave_named_scope("proj", _ps_[0], False)

    nc.finalize()
    return nc


_NC = None


def _get_nc():
    global _NC
    if _NC is None:
        _NC = build_nc()
    return _NC


def _run(x, W_qkv, W_proj, b_proj, trace=False):
    nc = _get_nc()
    x = np.asarray(x).astype(ml_dtypes.bfloat16)
    W_qkv = np.ascontiguousarray(np.asarray(W_qkv).astype(ml_dtypes.bfloat16))
    W_proj = np.ascontiguousarray(np.asarray(W_proj).astype(ml_dtypes.bfloat16))
    b_bc = np.ascontiguousarray(
        np.broadcast_to(np.asarray(b_proj, dtype=np.float32), (128, C)))
    in_maps = [
        {
            "xT": np.ascontiguousarray(x[i].T),
            "W_qkv": W_qkv,
            "W_proj": W_proj,
            "b_bc": b_bc,
        }
        for i in range(N_CORES)
    ]
    res = run_bass_kernel_spmd(nc, in_maps, core_ids=list(range(N_CORES)),
                               trace=trace)
    out = np.stack([res.results[i]["out"] for i in range(N_CORES)], axis=0)
    return out.astype(np.float32), res


def kernel(x, W_qkv, W_proj, b_proj):
    out, _ = _run(x, W_qkv, W_proj, b_proj, trace=False)
    return out


# revision 29
# speedup vs baseline: 1.1865x; 1.1865x over previous
"""Trainium2 Bass kernel for batch-8 multi-head attention (B=8, N=1024, C=768, H=12).

Distribution: pure data parallelism — batch element i runs entirely on core i
(weights replicated, zero collectives, full inputs sharded on host).

Host-side prep inside kernel(): inputs cast to bf16, x pre-transposed per
core, bias pre-broadcast to [128, C], so xT / W_qkv / W_proj / b DMA straight
into their SBUF layouts with no on-device conversion or PE transposes; the
output DMAs out as bf16.

Per-core pipeline:
  xT[k, t]      DMA'd directly (host-transposed), chunk k = cols [k*N, ...)
  qT/kT         W_qkv col-block (stationary) x xT (moving) -> 24 per-head
                blocks [128, N]; rows 64:127 zeroed (Pool memsets) so scores
                stream a full 128-wide contraction (both operands must be
                NaN-free there: 0*garbage = NaN)
  v[t, c]       xT chunk (stationary) x W_v (moving), stored as H blocks of
                [v_h(64) | 1] per s-tile (v65 memset to 1.0 once)
  ST[s, t]      kT slice (stationary) x qT (moving)           (scores^T)
  expST         ACT exp(SCALE * ST) PSUM->SBUF bf16; ACT does almost nothing
                else so the 96 exps stream densely
  y65[t, d|cs]  expST slice (stationary) x [v|1] (moving); col 64 = softmax
                denominator -> DVE reciprocal + tensor_scalar normalize
  yT            PE transposes of y_nat (only transposes left on the PE)
  z[t, c]       split-K projection: chunks 0-2 (heads 0-5) projected during
                heads 9-12 into z1 = psum + bias; chunks 3-5 at the end,
                z = psum + z1, DMA'd per t-tile as soon as ready

Emission interleaves scores s-tiles of head h with AV t-tiles of head h-3
(exp pool = 24 tiles = 3 heads) plus one late qk column per head, keeping
the in-order PE queue from parking useful work behind an exp-paced wait.
"""
import numpy as np
import ml_dtypes

import concourse.bacc as bacc
import concourse.bass as bass
import concourse.tile as tile
import concourse.mybir as mybir
from concourse import masks
from concourse.bass_utils import run_bass_kernel_spmd

F32 = mybir.dt.float32
BF16 = mybir.dt.bfloat16

B, N, C = 8, 1024, 768
H, D = 12, 64
SCALE = float(D) ** -0.5
N_CORES = 8
KT = C // 128             # 6 contraction chunks of 128
TT = N // 128             # 8 token tiles of 128
ST = N // 128             # 8 key tiles of 128
EXP_FN = mybir.ActivationFunctionType.Exp


def halves(width):
    out = []
    off = 0
    while off < width:
        w = min(512, width - off)
        out.append((off, w))
        off += w
    return out


def build_nc():
    nc = bacc.Bacc("TRN2", target_bir_lowering=False, debug=False,
                   num_devices=N_CORES)
    xt_ext = nc.dram_tensor("xT", [C, N], BF16, kind="ExternalInput")
    wqkv_ext = nc.dram_tensor("W_qkv", [C, 3 * C], BF16, kind="ExternalInput")
    wproj_ext = nc.dram_tensor("W_proj", [C, C], BF16, kind="ExternalInput")
    bb_ext = nc.dram_tensor("b_bc", [128, C], F32, kind="ExternalInput")
    out_ext = nc.dram_tensor("out", [N, C], BF16, kind="ExternalOutput")

    with tile.TileContext(nc) as tc:
        with (
            tc.tile_pool(name="const", bufs=1) as constp,
            tc.tile_pool(name="wq", bufs=1) as wqp,
            tc.tile_pool(name="xt", bufs=1) as xtp,
            tc.tile_pool(name="qk", bufs=1) as qkp,
            tc.tile_pool(name="vp", bufs=1) as vp,
            tc.tile_pool(name="yt", bufs=1) as ytp,
            tc.tile_pool(name="yn", bufs=1) as ynp,
            tc.tile_pool(name="z1p", bufs=1) as z1p,
            tc.tile_pool(name="recip", bufs=1) as recipp,
            tc.tile_pool(name="exp", bufs=24) as expp,
            tc.tile_pool(name="z", bufs=2) as zp,
            tc.tile_pool(name="psum", bufs=3, space="PSUM") as psum,
        ):
            # ---- persistent tensors ----
            xt_bf = xtp.tile([128, KT * N], BF16)          # xT: chunk k at cols [k*N, ...)
            wq_bf = wqp.tile([128, KT * 3 * C], BF16)      # W_qkv chunk k at cols [k*3C, ...)
            wp_bf = wqp.tile([128, KT * C], BF16)          # W_proj chunk k at cols [k*C, ...)
            qk_pad = qkp.tile([128, 24 * N], BF16)         # qT blocks 0..11, kT blocks 12..23
            v65 = vp.tile([128, ST * H * 65], BF16)        # per s-tile: H blocks of [v_h(64)|1]
            y_nat = ynp.tile([128, TT * C], BF16)          # y natural: t-tile t at cols [t*C, ...)
            yt_bf = ytp.tile([128, KT * N], BF16)          # yT: chunk c at cols [c*N, ...)
            z1_sb = z1p.tile([128, TT * C], BF16)          # proj half 1 partial (incl bias)

            # ---- constants ----
            ident = constp.tile([128, 128], BF16)
            masks.make_identity(nc, ident[:])
            b_bcast = constp.tile([128, C], F32)

            # ---- DMAs: xT + W_proj (+ outputs later) on sync queue, W_qkv
            # + bias on the scalar queue ----
            for k in range(KT):
                nc.sync.dma_start(xt_bf[:, k * N:(k + 1) * N],
                                  xt_ext[k * 128:(k + 1) * 128, :])
            for k in range(KT):
                nc.scalar.dma_start(wq_bf[:, k * 3 * C:(k + 1) * 3 * C],
                                    wqkv_ext[k * 128:(k + 1) * 128, :])
            for k in range(KT):
                nc.sync.dma_start(wp_bf[:, k * C:(k + 1) * C],
                                  wproj_ext[k * 128:(k + 1) * 128, :])
            nc.scalar.dma_start(b_bcast[:], bb_ext[:])

            # ---- zero pad rows + v65 ones (Pool; SBUF-only engine) ----
            # blocks for heads 0/1 first so scores h0/h1 aren't gated on the
            # big sweep
            nc.gpsimd.memset(qk_pad[64:128, 0:2 * N], 0.0)
            nc.gpsimd.memset(qk_pad[64:128, 12 * N:14 * N], 0.0)
            nc.gpsimd.memset(v65[:], 1.0)
            nc.gpsimd.memset(qk_pad[64:128, 2 * N:7 * N], 0.0)
            nc.gpsimd.memset(qk_pad[64:128, 14 * N:19 * N], 0.0)
            nc.gpsimd.memset(qk_pad[64:128, 7 * N:12 * N], 0.0)
            nc.gpsimd.memset(qk_pad[64:128, 19 * N:24 * N], 0.0)

            # ---- emitters ----
            def emit_qk_col(co, ev0, ev1):
                # co 0..5: q pair (heads 2co, 2co+1) -> blocks 2co, 2co+1
                # co 6..11: k pair -> blocks 12+2(co-6), 13+2(co-6)
                woff = co * 128 if co < KT else C + (co - KT) * 128
                qk_ps = psum.tile([128, N], F32, tag="ps")
                for k in range(KT):
                    lhsT = wq_bf[:, k * 3 * C + woff: k * 3 * C + woff + 128]
                    for off, w in halves(N):
                        nc.tensor.matmul(qk_ps[:, off:off + w], lhsT,
                                         xt_bf[:, k * N + off: k * N + off + w],
                                         start=(k == 0), stop=(k == KT - 1))
                if co < KT:
                    blk0, blk1 = 2 * co, 2 * co + 1
                else:
                    blk0, blk1 = 12 + 2 * (co - KT), 13 + 2 * (co - KT)
                ev0(qk_pad[0:64, blk0 * N:(blk0 + 1) * N], qk_ps[0:64, :])
                ev1(qk_pad[0:64, blk1 * N:(blk1 + 1) * N], qk_ps[64:128, :])

            def emit_v_tile(t):
                v_ps = psum.tile([128, C], F32, tag="ps")
                for k in range(KT):
                    lhsT = xt_bf[:, k * N + t * 128: k * N + (t + 1) * 128]
                    for off, w in halves(C):
                        nc.tensor.matmul(v_ps[:, off:off + w], lhsT,
                                         wq_bf[:, k * 3 * C + 2 * C + off: k * 3 * C + 2 * C + off + w],
                                         start=(k == 0), stop=(k == KT - 1))
                base = t * H * 65
                v_view = v65[:, base: base + H * 65].rearrange("p (h w) -> p h w", w=65)
                nc.vector.tensor_copy(v_view[:, :, 0:64],
                                      v_ps[:].rearrange("p (h d) -> p h d", d=64))

            e_tiles = {}

            def emit_score(h, s):
                k_ap = qk_pad[:, (12 + h) * N:(12 + h + 1) * N]
                q_ap = qk_pad[:, h * N:(h + 1) * N]
                s_ps = psum.tile([128, N], F32, tag="ps")
                for off, w in halves(N):
                    nc.tensor.matmul(s_ps[:, off:off + w],
                                     k_ap[:, s * 128:(s + 1) * 128],
                                     q_ap[:, off:off + w],
                                     start=True, stop=True)
                e_t = expp.tile([128, N], BF16, tag="exp")
                nc.scalar.activation(e_t[:], s_ps[:], EXP_FN, bias=0.0, scale=SCALE)
                e_tiles.setdefault(h, []).append(e_t)

            def emit_av(h, t):
                tiles = e_tiles[h]
                y_ps = psum.tile([128, 65], F32, tag="tp", bufs=2)
                for s in range(ST):
                    lhsT = tiles[s][:, t * 128:(t + 1) * 128]
                    rhs = v65[:, s * H * 65 + h * 65: s * H * 65 + (h + 1) * 65]
                    nc.tensor.matmul(y_ps[:, 0:65], lhsT, rhs,
                                     start=(s == 0), stop=(s == ST - 1))
                recip = recipp.tile([128, 1], F32, tag="recip", bufs=4)
                nc.vector.reciprocal(recip[:, 0:1], y_ps[:, 64:65])
                dst = y_nat[:, t * C + h * 64: t * C + (h + 1) * 64]
                nc.vector.tensor_scalar_mul(dst, y_ps[:, 0:64], recip[:, 0:1])
                if t == TT - 1:
                    e_tiles.pop(h)

            def emit_ytrans(i):
                # transpose y_nat c-chunk i (heads 2i, 2i+1) into yt_bf
                tp_ps = psum.tile([128, N], BF16, tag="tp", bufs=2)
                for t in range(TT):
                    nc.tensor.transpose(
                        tp_ps[:, t * 128:(t + 1) * 128],
                        y_nat[:, t * C + i * 128: t * C + (i + 1) * 128],
                        ident[:])
                nc.vector.tensor_copy(yt_bf[:, i * N:(i + 1) * N], tp_ps[:])

            def emit_proj(t, half):
                ks = (0, 1, 2) if half == 0 else (3, 4, 5)
                z_ps = psum.tile([128, C], F32, tag="ps")
                for k in ks:
                    lhsT = yt_bf[:, k * N + t * 128: k * N + (t + 1) * 128]
                    for off, w in halves(C):
                        nc.tensor.matmul(z_ps[:, off:off + w], lhsT,
                                         wp_bf[:, k * C + off: k * C + off + w],
                                         start=(k == ks[0]), stop=(k == ks[-1]))
                if half == 0:
                    nc.vector.tensor_add(z1_sb[:, t * C:(t + 1) * C], z_ps[:],
                                         b_bcast[:])
                else:
                    z_sb = zp.tile([128, C], BF16, tag="z")
                    nc.vector.tensor_add(z_sb[:], z_ps[:],
                                         z1_sb[:, t * C:(t + 1) * C])
                    nc.sync.dma_start(out_ext[t * 128:(t + 1) * 128, :], z_sb[:])

            # ---- qkv warm-up: cols for heads 0-3, scores h0-h2, v tiles ----
            _qs = nc.enter_named_scope("qkv", False)
            V = nc.vector.tensor_copy
            A = nc.scalar.copy
            emit_qk_col(0, A, A)
            emit_qk_col(6, A, A)
            for s in range(ST):
                emit_score(0, s)
            emit_qk_col(1, V, V)
            emit_qk_col(7, V, V)
            for s in range(ST):
                emit_score(1, s)
            for t in range(0, 3):
                emit_v_tile(t)
            for s in range(ST):
                emit_score(2, s)
            for t in range(3, 6):
                emit_v_tile(t)
            nc.leave_named_scope("qkv", _qs[0], False)

            # ---- attention: interleaved scores(h) / AV(h-3) / extras ----
            # (exp pool = 24 tiles = 3 heads, so AV trails scores by 3)
            _as = nc.enter_named_scope("attn", False)
            emit_v_tile(6)
            emit_v_tile(7)
            emit_qk_col(2, V, V)
            late_cols = [8, 3, 9, 4, 10, 5, 11]
            proj1_t = 0
            for h in range(3, H + 2):
                hs = h if h < H else None
                avs = [0, 1] if h == 3 else [h - 2]
                av_pairs = [(a, t) for a in avs for t in range(TT)]
                extras = []
                if h - 3 < len(late_cols):
                    extras.append(("col", late_cols[h - 3]))
                if h >= 8 and proj1_t < TT:
                    extras.append(("p", proj1_t))
                    extras.append(("p", proj1_t + 1))
                    proj1_t += 2
                # ytrans(i) once AV for head pair (2i, 2i+1) is done
                yt_is = [a // 2 for a in avs if a % 2 == 1]

                def run_extra(e):
                    kind, arg = e
                    if kind == "col":
                        emit_qk_col(arg, V, V)
                    else:
                        emit_proj(arg, 0)

                ai = iter(av_pairs)

                def take2():
                    for p in (next(ai, None), next(ai, None)):
                        if p:
                            emit_av(*p)

                if hs is not None:
                    for s in range(3):
                        emit_score(hs, s)
                    take2()
                    emit_score(hs, 3)
                    take2()
                    emit_score(hs, 4)
                    take2()
                    emit_score(hs, 5)
                    if extras:
                        run_extra(extras.pop(0))
                    emit_score(hs, 6)
                    take2()
                    emit_score(hs, 7)
                    for p in ai:
                        emit_av(*p)
                    for e in extras:
                        run_extra(e)
                else:
                    for p in ai:
                        emit_av(*p)
                    for e in extras:
                        run_extra(e)
                for i in yt_is:
                    emit_ytrans(i)
            nc.leave_named_scope("attn", _as[0], False)

            # ---- projection half 2 + output ----
            _ps_ = nc.enter_named_scope("proj", False)
            for t in range(TT):
                emit_proj(t, 1)
            nc.leave_named_scope("proj", _ps_[0], False)

    nc.finalize()
    return nc


_NC = None


def _get_nc():
    global _NC
    if _NC is None:
        _NC = build_nc()
    return _NC


def _run(x, W_qkv, W_proj, b_proj, trace=False):
    nc = _get_nc()
    x = np.asarray(x).astype(ml_dtypes.bfloat16)
    W_qkv = np.ascontiguousarray(np.asarray(W_qkv).astype(ml_dtypes.bfloat16))
    W_proj = np.ascontiguousarray(np.asarray(W_proj).astype(ml_dtypes.bfloat16))
    b_bc = np.ascontiguousarray(
        np.broadcast_to(np.asarray(b_proj, dtype=np.float32), (128, C)))
    in_maps = [
        {
            "xT": np.ascontiguousarray(x[i].T),
            "W_qkv": W_qkv,
            "W_proj": W_proj,
            "b_bc": b_bc,
        }
        for i in range(N_CORES)
    ]
    res = run_bass_kernel_spmd(nc, in_maps, core_ids=list(range(N_CORES)),
                               trace=trace)
    out = np.stack([res.results[i]["out"] for i in range(N_CORES)], axis=0)
    return out.astype(np.float32), res


def kernel(x, W_qkv, W_proj, b_proj):
    out, _ = _run(x, W_qkv, W_proj, b_proj, trace=False)
    return out


# revision 30
# speedup vs baseline: 1.1984x; 1.0100x over previous
"""Trainium2 Bass kernel for batch-8 multi-head attention (B=8, N=1024, C=768, H=12).

Distribution: pure data parallelism — batch element i runs entirely on core i
(weights replicated, zero collectives, full inputs sharded on host).

Host-side prep inside kernel(): inputs cast to bf16, x pre-transposed per
core, bias pre-broadcast to [128, C], so xT / W_qkv / W_proj / b DMA straight
into their SBUF layouts with no on-device conversion or PE transposes; the
output DMAs out as bf16.

Per-core pipeline:
  xT[k, t]      DMA'd directly (host-transposed), chunk k = cols [k*N, ...)
  qT/kT         W_qkv col-block (stationary) x xT (moving) -> 24 per-head
                blocks [128, N]; rows 64:127 zeroed (Pool memsets) so scores
                stream a full 128-wide contraction (both operands must be
                NaN-free there: 0*garbage = NaN)
  v[t, c]       xT chunk (stationary) x W_v (moving), stored as H blocks of
                [v_h(64) | 1] per s-tile (v65 memset to 1.0 once)
  ST[s, t]      kT slice (stationary) x qT (moving)           (scores^T)
  expST         ACT exp(SCALE * ST) PSUM->SBUF bf16; ACT does almost nothing
                else so the 96 exps stream densely
  y65[t, d|cs]  expST slice (stationary) x [v|1] (moving); col 64 = softmax
                denominator -> DVE reciprocal + tensor_scalar normalize
  yT            PE transposes of y_nat (only transposes left on the PE)
  z[t, c]       split-K projection: chunks 0-2 (heads 0-5) projected during
                heads 9-12 into z1 = psum + bias; chunks 3-5 at the end,
                z = psum + z1, DMA'd per t-tile as soon as ready

Emission interleaves scores s-tiles of head h with AV t-tiles of head h-3
(exp pool = 24 tiles = 3 heads) plus one late qk column per head, keeping
the in-order PE queue from parking useful work behind an exp-paced wait.
"""
import numpy as np
import ml_dtypes

import concourse.bacc as bacc
import concourse.bass as bass
import concourse.tile as tile
import concourse.mybir as mybir
from concourse import masks
from concourse.bass_utils import run_bass_kernel_spmd

F32 = mybir.dt.float32
BF16 = mybir.dt.bfloat16

B, N, C = 8, 1024, 768
H, D = 12, 64
SCALE = float(D) ** -0.5
N_CORES = 8
KT = C // 128             # 6 contraction chunks of 128
TT = N // 128             # 8 token tiles of 128
ST = N // 128             # 8 key tiles of 128
EXP_FN = mybir.ActivationFunctionType.Exp


def halves(width):
    out = []
    off = 0
    while off < width:
        w = min(512, width - off)
        out.append((off, w))
        off += w
    return out


def build_nc():
    nc = bacc.Bacc("TRN2", target_bir_lowering=False, debug=False,
                   num_devices=N_CORES)
    xt_ext = nc.dram_tensor("xT", [C, N], BF16, kind="ExternalInput")
    wqkv_ext = nc.dram_tensor("W_qkv", [C, 3 * C], BF16, kind="ExternalInput")
    wproj_ext = nc.dram_tensor("W_proj", [C, C], BF16, kind="ExternalInput")
    bb_ext = nc.dram_tensor("b_bc", [128, C], F32, kind="ExternalInput")
    out_ext = nc.dram_tensor("out", [N, C], BF16, kind="ExternalOutput")

    with tile.TileContext(nc) as tc:
        with (
            tc.tile_pool(name="const", bufs=1) as constp,
            tc.tile_pool(name="wq", bufs=1) as wqp,
            tc.tile_pool(name="xt", bufs=1) as xtp,
            tc.tile_pool(name="qk", bufs=1) as qkp,
            tc.tile_pool(name="vp", bufs=1) as vp,
            tc.tile_pool(name="yt", bufs=1) as ytp,
            tc.tile_pool(name="yn", bufs=1) as ynp,
            tc.tile_pool(name="z1p", bufs=1) as z1p,
            tc.tile_pool(name="recip", bufs=1) as recipp,
            tc.tile_pool(name="exp", bufs=24) as expp,
            tc.tile_pool(name="z", bufs=2) as zp,
            tc.tile_pool(name="psum", bufs=3, space="PSUM") as psum,
        ):
            # ---- persistent tensors ----
            xt_bf = xtp.tile([128, KT * N], BF16)          # xT: chunk k at cols [k*N, ...)
            wq_bf = wqp.tile([128, KT * 3 * C], BF16)      # W_qkv chunk k at cols [k*3C, ...)
            wp_bf = wqp.tile([128, KT * C], BF16)          # W_proj chunk k at cols [k*C, ...)
            qk_pad = qkp.tile([128, 24 * N], BF16)         # qT blocks 0..11, kT blocks 12..23
            v65 = vp.tile([128, ST * H * 65], BF16)        # per s-tile: H blocks of [v_h(64)|1]
            y_nat = ynp.tile([128, TT * C], BF16)          # y natural: t-tile t at cols [t*C, ...)
            yt_bf = ytp.tile([128, KT * N], BF16)          # yT: chunk c at cols [c*N, ...)
            z1_sb = z1p.tile([128, TT * C], BF16)          # proj half 1 partial (incl bias)

            # ---- constants ----
            ident = constp.tile([128, 128], BF16)
            masks.make_identity(nc, ident[:])
            b_bcast = constp.tile([128, C], F32)

            # ---- DMAs: xT + W_proj (+ outputs later) on sync queue, W_qkv
            # + bias on the scalar queue ----
            for k in range(KT):
                nc.sync.dma_start(xt_bf[:, k * N:(k + 1) * N],
                                  xt_ext[k * 128:(k + 1) * 128, :])
            for k in range(KT):
                nc.scalar.dma_start(wq_bf[:, k * 3 * C:(k + 1) * 3 * C],
                                    wqkv_ext[k * 128:(k + 1) * 128, :])
            for k in range(KT):
                nc.sync.dma_start(wp_bf[:, k * C:(k + 1) * C],
                                  wproj_ext[k * 128:(k + 1) * 128, :])
            nc.scalar.dma_start(b_bcast[:], bb_ext[:])

            # ---- zero pad rows + v65 ones (Pool; SBUF-only engine) ----
            # blocks for heads 0/1 first so scores h0/h1 aren't gated on the
            # big sweep
            nc.gpsimd.memset(qk_pad[64:128, 0:2 * N], 0.0)
            nc.gpsimd.memset(qk_pad[64:128, 12 * N:14 * N], 0.0)
            nc.gpsimd.memset(v65[:], 1.0)
            nc.gpsimd.memset(qk_pad[64:128, 2 * N:7 * N], 0.0)
            nc.gpsimd.memset(qk_pad[64:128, 14 * N:19 * N], 0.0)
            nc.gpsimd.memset(qk_pad[64:128, 7 * N:12 * N], 0.0)
            nc.gpsimd.memset(qk_pad[64:128, 19 * N:24 * N], 0.0)

            # ---- emitters ----
            def emit_qk_col(co, ev0, ev1):
                # co 0..5: q pair (heads 2co, 2co+1) -> blocks 2co, 2co+1
                # co 6..11: k pair -> blocks 12+2(co-6), 13+2(co-6)
                woff = co * 128 if co < KT else C + (co - KT) * 128
                qk_ps = psum.tile([128, N], F32, tag="ps")
                for k in range(KT):
                    lhsT = wq_bf[:, k * 3 * C + woff: k * 3 * C + woff + 128]
                    for off, w in halves(N):
                        nc.tensor.matmul(qk_ps[:, off:off + w], lhsT,
                                         xt_bf[:, k * N + off: k * N + off + w],
                                         start=(k == 0), stop=(k == KT - 1))
                if co < KT:
                    blk0, blk1 = 2 * co, 2 * co + 1
                else:
                    blk0, blk1 = 12 + 2 * (co - KT), 13 + 2 * (co - KT)
                ev0(qk_pad[0:64, blk0 * N:(blk0 + 1) * N], qk_ps[0:64, :])
                ev1(qk_pad[0:64, blk1 * N:(blk1 + 1) * N], qk_ps[64:128, :])

            def emit_v_tile(t):
                v_ps = psum.tile([128, C], F32, tag="ps")
                for k in range(KT):
                    lhsT = xt_bf[:, k * N + t * 128: k * N + (t + 1) * 128]
                    for off, w in halves(C):
                        nc.tensor.matmul(v_ps[:, off:off + w], lhsT,
                                         wq_bf[:, k * 3 * C + 2 * C + off: k * 3 * C + 2 * C + off + w],
                                         start=(k == 0), stop=(k == KT - 1))
                base = t * H * 65
                v_view = v65[:, base: base + H * 65].rearrange("p (h w) -> p h w", w=65)
                nc.vector.tensor_copy(v_view[:, :, 0:64],
                                      v_ps[:].rearrange("p (h d) -> p h d", d=64))

            e_tiles = {}

            def emit_score(h, s):
                k_ap = qk_pad[:, (12 + h) * N:(12 + h + 1) * N]
                q_ap = qk_pad[:, h * N:(h + 1) * N]
                s_ps = psum.tile([128, N], F32, tag="ps")
                for off, w in halves(N):
                    nc.tensor.matmul(s_ps[:, off:off + w],
                                     k_ap[:, s * 128:(s + 1) * 128],
                                     q_ap[:, off:off + w],
                                     start=True, stop=True)
                e_t = expp.tile([128, N], BF16, tag="exp")
                nc.scalar.activation(e_t[:], s_ps[:], EXP_FN, bias=0.0, scale=SCALE)
                e_tiles.setdefault(h, []).append(e_t)

            def emit_av(h, t):
                tiles = e_tiles[h]
                y_ps = psum.tile([128, 65], F32, tag="tp", bufs=2)
                for s in range(ST):
                    lhsT = tiles[s][:, t * 128:(t + 1) * 128]
                    rhs = v65[:, s * H * 65 + h * 65: s * H * 65 + (h + 1) * 65]
                    nc.tensor.matmul(y_ps[:, 0:65], lhsT, rhs,
                                     start=(s == 0), stop=(s == ST - 1))
                recip = recipp.tile([128, 1], F32, tag="recip", bufs=4)
                nc.vector.reciprocal(recip[:, 0:1], y_ps[:, 64:65])
                dst = y_nat[:, t * C + h * 64: t * C + (h + 1) * 64]
                nc.vector.tensor_scalar_mul(dst, y_ps[:, 0:64], recip[:, 0:1])
                if t == TT - 1:
                    e_tiles.pop(h)

            def emit_ytrans(i):
                # transpose y_nat c-chunk i (heads 2i, 2i+1) into yt_bf
                tp_ps = psum.tile([128, N], BF16, tag="tp", bufs=2)
                for t in range(TT):
                    nc.tensor.transpose(
                        tp_ps[:, t * 128:(t + 1) * 128],
                        y_nat[:, t * C + i * 128: t * C + (i + 1) * 128],
                        ident[:])
                nc.vector.tensor_copy(yt_bf[:, i * N:(i + 1) * N], tp_ps[:])

            def emit_proj(t, half):
                ks = (0, 1, 2) if half == 0 else (3, 4, 5)
                z_ps = psum.tile([128, C], F32, tag="ps")
                for k in ks:
                    lhsT = yt_bf[:, k * N + t * 128: k * N + (t + 1) * 128]
                    for off, w in halves(C):
                        nc.tensor.matmul(z_ps[:, off:off + w], lhsT,
                                         wp_bf[:, k * C + off: k * C + off + w],
                                         start=(k == ks[0]), stop=(k == ks[-1]))
                if half == 0:
                    nc.vector.tensor_add(z1_sb[:, t * C:(t + 1) * C], z_ps[:],
                                         b_bcast[:])
                else:
                    z_sb = zp.tile([128, C], BF16, tag="z")
                    nc.vector.tensor_add(z_sb[:], z_ps[:],
                                         z1_sb[:, t * C:(t + 1) * C])
                    q = nc.sync if t % 2 == 0 else nc.scalar
                    q.dma_start(out_ext[t * 128:(t + 1) * 128, :], z_sb[:])

            # ---- qkv warm-up: cols for heads 0-3, scores h0-h2, v tiles ----
            _qs = nc.enter_named_scope("qkv", False)
            V = nc.vector.tensor_copy
            A = nc.scalar.copy
            emit_qk_col(0, A, A)
            emit_qk_col(6, A, A)
            for s in range(ST):
                emit_score(0, s)
            emit_qk_col(1, V, V)
            emit_qk_col(7, V, V)
            for s in range(ST):
                emit_score(1, s)
            for t in range(0, 3):
                emit_v_tile(t)
            for s in range(ST):
                emit_score(2, s)
            for t in range(3, 6):
                emit_v_tile(t)
            nc.leave_named_scope("qkv", _qs[0], False)

            # ---- attention: interleaved scores(h) / AV(h-3) / extras ----
            # (exp pool = 24 tiles = 3 heads, so AV trails scores by 3)
            _as = nc.enter_named_scope("attn", False)
            emit_v_tile(6)
            emit_v_tile(7)
            emit_qk_col(2, V, V)
            late_cols = [8, 3, 9, 4, 10, 5, 11]
            proj1_t = 0
            for h in range(3, H + 2):
                hs = h if h < H else None
                avs = [0, 1] if h == 3 else [h - 2]
                av_pairs = [(a, t) for a in avs for t in range(TT)]
                extras = []
                if h - 3 < len(late_cols):
                    extras.append(("col", late_cols[h - 3]))
                if h >= 8 and proj1_t < TT:
                    extras.append(("p", proj1_t))
                    extras.append(("p", proj1_t + 1))
                    proj1_t += 2
                # ytrans(i) once AV for head pair (2i, 2i+1) is done
                yt_is = [a // 2 for a in avs if a % 2 == 1]

                def run_extra(e):
                    kind, arg = e
                    if kind == "col":
                        emit_qk_col(arg, V, V)
                    else:
                        emit_proj(arg, 0)

                ai = iter(av_pairs)

                def take2():
                    for p in (next(ai, None), next(ai, None)):
                        if p:
                            emit_av(*p)

                if hs is not None:
                    for s in range(3):
                        emit_score(hs, s)
                    take2()
                    emit_score(hs, 3)
                    take2()
                    emit_score(hs, 4)
                    take2()
                    emit_score(hs, 5)
                    if extras:
                        run_extra(extras.pop(0))
                    emit_score(hs, 6)
                    take2()
                    emit_score(hs, 7)
                    for p in ai:
                        emit_av(*p)
                    for e in extras:
                        run_extra(e)
                else:
                    for p in ai:
                        emit_av(*p)
                    for e in extras:
                        run_extra(e)
                for i in yt_is:
                    emit_ytrans(i)
            nc.leave_named_scope("attn", _as[0], False)

            # ---- projection half 2 + output ----
            _ps_ = nc.enter_named_scope("proj", False)
            for t in range(TT):
                emit_proj(t, 1)
            nc.leave_named_scope("proj", _ps_[0], False)

    nc.finalize()
    return nc


_NC = None


def _get_nc():
    global _NC
    if _NC is None:
        _NC = build_nc()
    return _NC


def _run(x, W_qkv, W_proj, b_proj, trace=False):
    nc = _get_nc()
    x = np.asarray(x).astype(ml_dtypes.bfloat16)
    W_qkv = np.ascontiguousarray(np.asarray(W_qkv).astype(ml_dtypes.bfloat16))
    W_proj = np.ascontiguousarray(np.asarray(W_proj).astype(ml_dtypes.bfloat16))
    b_bc = np.ascontiguousarray(
        np.broadcast_to(np.asarray(b_proj, dtype=np.float32), (128, C)))
    in_maps = [
        {
            "xT": np.ascontiguousarray(x[i].T),
            "W_qkv": W_qkv,
            "W_proj": W_proj,
            "b_bc": b_bc,
        }
        for i in range(N_CORES)
    ]
    res = run_bass_kernel_spmd(nc, in_maps, core_ids=list(range(N_CORES)),
                               trace=trace)
    out = np.stack([res.results[i]["out"] for i in range(N_CORES)], axis=0)
    return out.astype(np.float32), res


def kernel(x, W_qkv, W_proj, b_proj):
    out, _ = _run(x, W_qkv, W_proj, b_proj, trace=False)
    return out
